# revision 19
# baseline (speedup 1.0000x reference)
"""Multi-head causal self-attention (B=4, S=2048, E=1024, H=16, Dh=64) on 8
Trainium2 NeuronCores.

Sharding: tensor-parallel over heads - 2 heads per core. Each core computes
q/k/v projections, causal attention and its slice of the output projection
(rows of Wo for its heads); the host sums the 8 partial outputs and adds bo.

v2 design (vs the fp32r baseline):
  - bf16 everywhere on the PE (inputs host-cast): transposes run at 1
    cycle/row (vs 2 for fp32), DMA traffic and DVE copies are halved, and
    PSUM transpose staging fits in half-banks. PSUM accumulation stays fp32.
  - QK pair per key tile issued back-to-back as row-tiled matmuls (head 0 in
    PE rows 0-63, head 1 in rows 64-127, via base_partition auto tiling) so
    the two heads' scores co-run on the array; their outputs land in one
    [128, 2, 512] PSUM pair-tile and are exp'd by a single ACT instruction.
  - causal: key tiles above the diagonal are skipped entirely; on diagonal
    tiles the QK/exp streams are restricted to the unmasked query columns
    and gpsimd affine_select zero-fills the masked region of the exp tile.
  - softmax denominators ride as a ones-column appended to v (PV output row
    64); attention outputs are transposed to natural [token, dh] layout,
    normalized there with per-partition reciprocals, and transposed back
    packed [2*64, token] so the output projection runs as single K=128
    matmuls (full PE density, no post-scaling of z needed).
  - emission interleaves the next chunk's x-transposes/projections and the
    previous chunk's normalization + output projection into the attention
    key-tile stream, so the PE never sees a long transpose-only or
    DMA-wait window (transposes don't count as PE-busy for the HAM clock
    gate; an idle window re-throttles the PE to 1.2 GHz).
  - a short warmup matmul burst at kernel start brings the PE out of the
    cold 1.2 GHz state while the first DMAs land.

All PE transposes use full 128-partition operands: partial-partition
transposes interleaved with other PE work crash the device
(NRT_EXEC_UNIT_UNRECOVERABLE).
"""

import os

import ml_dtypes
import numpy as np

import concourse.bass as bass
import concourse.mybir as mybir
import concourse.tile as tile
from concourse.vector_clock import ScopedClock
from concourse.masks import make_identity
from concourse.bass_utils import run_bass_kernel_spmd

F32 = mybir.dt.float32
BF = mybir.dt.bfloat16
AF = mybir.ActivationFunctionType
ALU = mybir.AluOpType

B, S, E, H, DH = 4, 2048, 1024, 16, 64
NCORES = 8
HP = 2            # heads per core
SC = 512          # query chunk (columns of scoresT)
NSC = S // SC     # 4 chunks per batch
TT = 128          # key tile
NTT = S // TT     # 16 key tiles per batch
EO = E // 128     # 8 contraction chunks


class SafeTileContext(tile.TileContext):
    """TileContext with the tail drain's sem waits split across multiple
    Drain instructions - walrus here rejects >1 sync wait per instruction."""

    MAX_DRAIN_WAITS = 1

    def _drain_and_barrier(self, tick_clock, wait_clock):
        nc = self.nc
        drain_inst = nc.sync.drain()
        wait_clock.add_sem_waits(
            drain_inst.ins, ScopedClock({None: tick_clock.global_clock})
        )
        si = drain_inst.ins.sync_info
        if si is not None and si.on_wait and len(si.on_wait) > self.MAX_DRAIN_WAITS:
            waits = list(si.on_wait)
            si.on_wait = waits[: self.MAX_DRAIN_WAITS]
            drain_inst.ins.sync_info = si
            for i in range(self.MAX_DRAIN_WAITS, len(waits), self.MAX_DRAIN_WAITS):
                extra = nc.sync.drain()
                extra.ins.sync_info = mybir.SyncInfo(
                    on_wait=waits[i : i + self.MAX_DRAIN_WAITS], on_update=[]
                )
        nc.all_engine_barrier()
        assert self.sems is not None
        popped = nc._tile_sem_poison_stack.pop()
        assert popped is self._sem_poison
        nc.clear_and_free_semaphores(list(self.sems.allocated().values()))
        nc.all_engine_barrier()


def split_sync_waits(nc, maxw=1):
    """Hoist excess sync waits onto same-engine NoOps inserted just before
    the over-limit instruction (this container's walrus allows ~1)."""
    n_split = 0
    for f in nc.m.functions:
        for blk in f.blocks:
            out = []
            for ins in blk.instructions:
                si = ins.sync_info
                if si is not None and si.on_wait and len(si.on_wait) > maxw:
                    waits = list(si.on_wait)
                    extra, keep = waits[:-maxw], waits[-maxw:]
                    for j in range(0, len(extra), maxw):
                        nop = mybir.InstNoOp(
                            name=f"{ins.name}-wsplit{j}", ins=[], outs=[]
                        )
                        nop.engine = ins.engine
                        nop.sync_info = mybir.SyncInfo(
                            on_wait=extra[j : j + maxw], on_update=[]
                        )
                        out.append(nop)
                    si.on_wait = keep
                    ins.sync_info = si
                    n_split += 1
                out.append(ins)
            blk.instructions = out
    return n_split


def build_bass():
    nc = bass.Bass()
    x_d = nc.dram_tensor("x", [B, E, S], BF, kind="ExternalInput")  # pre-transposed on host
    wq_d = nc.dram_tensor("wq", [E, HP * DH], BF, kind="ExternalInput")
    wk_d = nc.dram_tensor("wk", [E, HP * DH], BF, kind="ExternalInput")
    wv_d = nc.dram_tensor("wv", [E, HP * DH], BF, kind="ExternalInput")
    bqkv_d = nc.dram_tensor("bqkv", [128, 3], F32, kind="ExternalInput")
    wo_d = nc.dram_tensor("wo", [HP * DH, E], BF, kind="ExternalInput")
    z_d = nc.dram_tensor("zpart", [B, S, E], BF, kind="ExternalOutput")

    with SafeTileContext(nc) as tc:
        with (
            tc.tile_pool(name="const", bufs=1) as constp,
            tc.tile_pool(name="xt", bufs=3) as xt_p,
            tc.tile_pool(name="qkv", bufs=2) as qkv_p,
            tc.tile_pool(name="vpool", bufs=2) as v_p,
            tc.tile_pool(name="vt", bufs=2) as vt_p,
            tc.tile_pool(name="expt", bufs=4) as exp_p,
            tc.tile_pool(name="oraw", bufs=4) as oraw_p,
            tc.tile_pool(name="onrm", bufs=2) as onrm_p,
            tc.tile_pool(name="osb", bufs=2) as osb_p,
            tc.tile_pool(name="den", bufs=2) as den_p,
            tc.tile_pool(name="zsb", bufs=4) as z_p,
            tc.tile_pool(name="ps_qk", bufs=1, space="PSUM") as ps_qk,
            tc.tile_pool(name="ps_po", bufs=2, space="PSUM") as ps_po,
            tc.tile_pool(name="ps_pw", bufs=2, space="PSUM") as ps_pw,
            tc.tile_pool(name="ps_tr", bufs=2, space="PSUM") as ps_tr,
        ):
            ident = constp.tile([128, 128], F32)
            make_identity(nc, ident)
            ident_bf = constp.tile([128, 128], BF)
            nc.vector.tensor_copy(out=ident_bf, in_=ident)

            # weights: [ei, eo, h*dh] stationary layout, bf16 straight from HBM
            w_sbs = []
            for nm, wd in (("wq", wq_d), ("wk", wk_d), ("wv", wv_d)):
                w_sb = constp.tile([128, EO, 128], BF, name=f"{nm}_sb")
                nc.sync.dma_start(
                    w_sb, wd.rearrange("(eo ei) d -> ei eo d", ei=128)
                )
                w_sbs.append(w_sb)
            wq_sb, wk_sb, wv_sb = w_sbs

            wo_sb = constp.tile([128, E], BF)
            nc.sync.dma_start(wo_sb, wo_d[:, :])

            bias_sb = constp.tile([128, 3], F32)
            nc.sync.dma_start(bias_sb, bqkv_d[:, :])

            ones_sb = constp.tile([128, 1], F32)
            nc.vector.memset(ones_sb, 1.0)

            # PE warmup: ~3us of matmuls so HAM reaches K=8/8 while the
            # first x/weight DMAs land.
            warm_sb = constp.tile([128, 512], BF)
            nc.vector.memset(warm_sb, 0.0)
            warm_ps = ps_pw.tile([128, 512], F32, tag="pw", name="warm_ps")
            for i in range(12):
                nc.tensor.matmul(
                    warm_ps, lhsT=warm_sb[:, 0:128], rhs=warm_sb,
                    start=(i == 0), stop=(i == 11),
                )

            batch_tiles = {}
            chunk_po = {}

            def head_slices(b, c):
                """Emission closures: x-transposes + q/k/v projections for
                chunk (b, c). 7 slices to interleave into attention."""
                s0 = c * SC

                def make_tiles():
                    qT = qkv_p.tile([128, S], BF, tag="qT", name=f"qT_{b}")
                    kT = qkv_p.tile([128, S], BF, tag="kT", name=f"kT_{b}")
                    v_sb = v_p.tile(
                        [128, HP, NTT, DH + 1], BF, tag="v", name=f"v_{b}"
                    )
                    nc.vector.tensor_copy(
                        out=v_sb[:, :, :, DH : DH + 1],
                        in_=ones_sb.to_broadcast([128, HP, NTT, 1]),
                    )
                    batch_tiles[b] = (qT, kT, v_sb)

                xt = xt_p.tile([128, EO, SC], BF, tag="xt", name=f"xt_{b}_{c}")

                def dma_slice():
                    if c == 0:
                        make_tiles()
                    nc.sync.dma_start(
                        xt,
                        x_d[b, :, s0 : s0 + SC].rearrange(
                            "(eo ei) s -> ei eo s", ei=128
                        ),
                    )

                def proj_slice(kind):
                    def run():
                        qT, kT, v_sb = batch_tiles[b]
                        w_sb = {"q": wq_sb, "k": wk_sb, "v": wv_sb}[kind]
                        psp = ps_pw.tile([128, 512], F32, tag="pw", name="psp")
                        for eo in range(EO):
                            nc.tensor.matmul(
                                psp,
                                lhsT=w_sb[:, eo, :],
                                rhs=xt[:, eo, :],
                                start=(eo == 0),
                                stop=(eo == EO - 1),
                            )
                        col = {"q": 0, "k": 1, "v": 2}[kind]
                        bias_ap = bias_sb[:, col : col + 1]
                        if kind == "q":
                            nc.scalar.activation(
                                qT[:, s0 : s0 + SC], psp, AF.Identity, bias=bias_ap
                            )
                        elif kind == "k":
                            nc.scalar.activation(
                                kT[:, s0 : s0 + SC], psp, AF.Identity, bias=bias_ap
                            )
                        else:
                            vt = vt_p.tile([128, SC], BF, tag="vt", name="vt")
                            nc.vector.tensor_scalar_add(vt, psp, bias_ap)
                            for vg in range(2):
                                pstv = ps_tr.tile(
                                    [128, 2, 128], BF, tag="tr", name="pstv"
                                )
                                for tl in range(2):
                                    nc.tensor.transpose(
                                        pstv[:, tl, :],
                                        vt[
                                            :,
                                            (vg * 2 + tl) * 128
                                            : (vg * 2 + tl + 1) * 128,
                                        ],
                                        ident_bf,
                                    )
                                pv4 = pstv.rearrange(
                                    "p a (q b) -> p a q b", q=HP
                                )
                                for h in range(HP):
                                    nc.vector.tensor_copy(
                                        out=v_sb[
                                            :, h,
                                            c * 4 + vg * 2 : c * 4 + vg * 2 + 2,
                                            0:DH,
                                        ],
                                        in_=pv4[:, :, h, :],
                                    )
                    return run

                return [dma_slice, proj_slice("q"), proj_slice("k"), proj_slice("v")]

            def z_slices(b, c):
                """Normalization + output projection for chunk (b, c),
                consuming the saved PV accumulators. 5 slices."""
                s0 = c * SC
                box = {}

                def norm_slice():
                    po = chunk_po.pop((b, c))
                    ot_raw = [
                        oraw_p.tile([128, SC], BF, tag="or", name=f"or{h}_{b}_{c}")
                        for h in range(HP)
                    ]
                    for h in range(HP):
                        nc.vector.tensor_copy(
                            out=ot_raw[h][0 : DH + 1, :], in_=po[h][:, :]
                        )
                    dnat = ps_tr.tile([128, 8, 128], BF, tag="tr", name="dnat")
                    for st in range(4):
                        for h in range(HP):
                            nc.tensor.transpose(
                                dnat[:, st * 2 + h, :],
                                ot_raw[h][:, st * 128 : (st + 1) * 128],
                                ident_bf,
                            )
                    den_sb = den_p.tile([128, 8, 1], F32, tag="den", name="den_sb")
                    nc.vector.reciprocal(den_sb, dnat[:, :, DH : DH + 1])
                    onorm = onrm_p.tile([128, 8, DH], BF, tag="on", name="onorm")
                    nc.vector.scalar_tensor_tensor(
                        out=onorm,
                        in0=dnat[:, :, 0:DH],
                        scalar=1.0,
                        in1=den_sb.to_broadcast([128, 8, DH]),
                        op0=ALU.mult,
                        op1=ALU.mult,
                    )
                    box["onorm"] = onorm

                def pack_slice():
                    onorm = box["onorm"]
                    otp = ps_tr.tile([128, 4, 128], BF, tag="tr", name="otp")
                    for st in range(4):
                        nc.tensor.transpose(
                            otp[:, st, :], onorm[:, st * 2 : (st + 1) * 2, :],
                            ident_bf,
                        )
                    ot_sb = osb_p.tile([128, 4, 128], BF, tag="os", name="ot_sb")
                    nc.vector.tensor_copy(out=ot_sb, in_=otp)
                    box["ot_sb"] = ot_sb

                def wo_slice(st):
                    def run():
                        ot_sb = box["ot_sb"]
                        for ec in range(E // 512):
                            pz = ps_pw.tile([128, 512], F32, tag="pw", name="pz")
                            nc.tensor.matmul(
                                pz,
                                lhsT=ot_sb[:, st, :],
                                rhs=wo_sb[:, ec * 512 : (ec + 1) * 512],
                                start=True,
                                stop=True,
                            )
                            zt = z_p.tile([128, 512], BF, tag="z", name="zt")
                            nc.vector.tensor_copy(out=zt, in_=pz)
                            nc.sync.dma_start(
                                z_d[
                                    b,
                                    s0 + st * 128 : s0 + (st + 1) * 128,
                                    ec * 512 : (ec + 1) * 512,
                                ],
                                zt,
                            )
                    return run

                def wo01():
                    pack_slice()
                    wo_slice(0)()

                return [norm_slice, wo01, wo_slice(1), wo_slice(2), wo_slice(3)]

            def att_emit(b, c, fillers):
                """QK -> exp -> mask -> PV key-tile stream for chunk (b, c),
                with filler slices interleaved to keep the PE dense."""
                s0 = c * SC
                nkt = 4 * (c + 1)
                qT, kT, v_sb = batch_tiles[b]
                po = [
                    ps_po.tile([DH + 1, 512], F32, tag="po", name=f"po{h}_{b}_{c}")
                    for h in range(HP)
                ]
                chunk_po[(b, c)] = po

                def emit_pv(tt, et):
                    j = tt - 4 * c
                    cs = 128 * j if j > 0 else 0
                    for h in range(HP):
                        nc.tensor.matmul(
                            po[h][:, cs:512],
                            lhsT=v_sb[:, h, tt, :],
                            rhs=et[:, h, cs:512],
                            start=(tt == 0),
                            stop=(tt == nkt - 1),
                            skip_group_check=True,
                        )

                prev = None
                emitted = 0
                for tt in range(nkt):
                    j = tt - 4 * c
                    cs = 128 * j if j > 0 else 0
                    pss = ps_qk.tile([128, HP, 512], F32, tag="qk", name="pss")
                    for h in range(HP):
                        nc.tensor.matmul(
                            pss[:, h, cs:512],
                            lhsT=kT[h * DH : (h + 1) * DH, tt * TT : (tt + 1) * TT],
                            rhs=qT[h * DH : (h + 1) * DH, s0 + cs : s0 + SC],
                            start=True,
                            stop=True,
                        )
                    et = exp_p.tile([128, HP, SC], BF, tag="et", name="et")
                    nc.scalar.activation(
                        et[:, :, cs:512], pss[:, :, cs:512], AF.Exp, scale=0.125
                    )
                    if j >= 0:
                        # diagonal tile: zero where t > s (also zero-fills
                        # the skipped columns below cs)
                        nc.gpsimd.affine_select(
                            out=et,
                            in_=et,
                            pattern=[[0, HP], [1, SC]],
                            compare_op=ALU.is_ge,
                            fill=0.0,
                            base=-128 * j,
                            channel_multiplier=-1,
                        )
                    if prev is not None:
                        emit_pv(*prev)
                    prev = (tt, et)
                    target = (len(fillers) * (tt + 1)) // nkt
                    while emitted < target:
                        fillers[emitted]()
                        emitted += 1
                emit_pv(*prev)
                while emitted < len(fillers):
                    fillers[emitted]()
                    emitted += 1

            chunks = [(b, c) for b in range(B) for c in range(NSC)]
            for sl in head_slices(*chunks[0]):
                sl()
            for i, (b, c) in enumerate(chunks):
                # hoisted ahead of the key-tile loop: the previous chunk's
                # normalization (frees its PV banks before this chunk's
                # first PV) and the next chunk's xt DMA (prefetch)
                pre = []
                fillers = []
                if i >= 1:
                    zs = z_slices(*chunks[i - 1])
                    pre.append(zs[0])
                    fillers += zs[1:]
                if i + 1 < len(chunks):
                    hs = head_slices(*chunks[i + 1])
                    pre.append(hs[0])
                    fillers += hs[1:]
                for p in pre:
                    p()
                att_emit(b, c, fillers)
            for sl in z_slices(*chunks[-1]):
                sl()

    split_sync_waits(nc, maxw=1)
    return nc


_NC_CACHE = None
_LAST_RESULTS = None


def _get_nc():
    global _NC_CACHE
    if _NC_CACHE is None:
        _NC_CACHE = build_bass()
    return _NC_CACHE


def kernel(x, Wq, Wk, Wv, bq, bk, bv, Wo, bo):
    nc = _get_nc()
    bf16 = ml_dtypes.bfloat16
    x_bf = np.ascontiguousarray(
        np.asarray(x, dtype=np.float32).astype(bf16).transpose(0, 2, 1)
    )
    in_maps = []
    for cidx in range(NCORES):
        h0 = HP * cidx
        bias = np.stack(
            [
                np.asarray(bq[h0 : h0 + HP], np.float32).reshape(HP * DH),
                np.asarray(bk[h0 : h0 + HP], np.float32).reshape(HP * DH),
                np.asarray(bv[h0 : h0 + HP], np.float32).reshape(HP * DH),
            ],
            axis=1,
        )
        wq_p = np.concatenate(
            [np.asarray(Wq[h0 + h], np.float32) for h in range(HP)], axis=1
        ).astype(bf16)
        wk_p = np.concatenate(
            [np.asarray(Wk[h0 + h], np.float32) for h in range(HP)], axis=1
        ).astype(bf16)
        wv_p = np.concatenate(
            [np.asarray(Wv[h0 + h], np.float32) for h in range(HP)], axis=1
        ).astype(bf16)
        in_maps.append(
            {
                "x": x_bf,
                "wq": np.ascontiguousarray(wq_p),
                "wk": np.ascontiguousarray(wk_p),
                "wv": np.ascontiguousarray(wv_p),
                "bqkv": np.ascontiguousarray(bias, np.float32),
                "wo": np.ascontiguousarray(
                    np.asarray(Wo[h0 * DH : (h0 + HP) * DH, :], np.float32).astype(
                        bf16
                    )
                ),
            }
        )
    kw = {}
    if os.environ.get("KERNEL_TRACE"):
        kw = dict(trace=True, tmpdir=os.environ.get("KERNEL_TRACE_DIR") or None)
    res = run_bass_kernel_spmd(nc, in_maps, core_ids=list(range(NCORES)), **kw)
    global _LAST_RESULTS
    _LAST_RESULTS = res
    acc = np.zeros((B, S, E), np.float64)
    for r in res.results:
        acc += r["zpart"].astype(np.float64)
    acc += np.asarray(bo, np.float64)[None, None, :]
    return acc.astype(np.float32)


# revision 20
# speedup vs baseline: 1.2140x; 1.2140x over previous
"""Multi-head causal self-attention (B=4, S=2048, E=1024, H=16, Dh=64) on 8
Trainium2 NeuronCores.

Sharding: tensor-parallel over heads - 2 heads per core. Each core computes
q/k/v projections, causal attention and its slice of the output projection
(rows of Wo for its heads); the host sums the 8 partial outputs and adds bo.

v2 design (vs the fp32r baseline):
  - bf16 everywhere on the PE (inputs host-cast): transposes run at 1
    cycle/row (vs 2 for fp32), DMA traffic and DVE copies are halved, and
    PSUM transpose staging fits in half-banks. PSUM accumulation stays fp32.
  - QK pair per key tile issued back-to-back as row-tiled matmuls (head 0 in
    PE rows 0-63, head 1 in rows 64-127, via base_partition auto tiling) so
    the two heads' scores co-run on the array; their outputs land in one
    [128, 2, 512] PSUM pair-tile and are exp'd by a single ACT instruction.
  - causal: key tiles above the diagonal are skipped entirely; on diagonal
    tiles the QK/exp streams are restricted to the unmasked query columns
    and gpsimd affine_select zero-fills the masked region of the exp tile.
  - softmax denominators ride as a ones-column appended to v (PV output row
    64); attention outputs are transposed to natural [token, dh] layout,
    normalized there with per-partition reciprocals, and transposed back
    packed [2*64, token] so the output projection runs as single K=128
    matmuls (full PE density, no post-scaling of z needed).
  - emission interleaves the next chunk's x-transposes/projections and the
    previous chunk's normalization + output projection into the attention
    key-tile stream, so the PE never sees a long transpose-only or
    DMA-wait window (transposes don't count as PE-busy for the HAM clock
    gate; an idle window re-throttles the PE to 1.2 GHz).
  - a short warmup matmul burst at kernel start brings the PE out of the
    cold 1.2 GHz state while the first DMAs land.

All PE transposes use full 128-partition operands: partial-partition
transposes interleaved with other PE work crash the device
(NRT_EXEC_UNIT_UNRECOVERABLE).
"""

import os

import ml_dtypes
import numpy as np

import concourse.bass as bass
import concourse.mybir as mybir
import concourse.tile as tile
from concourse.vector_clock import ScopedClock
from concourse.masks import make_identity
from concourse.bass_utils import run_bass_kernel_spmd

F32 = mybir.dt.float32
BF = mybir.dt.bfloat16
AF = mybir.ActivationFunctionType
ALU = mybir.AluOpType

B, S, E, H, DH = 4, 2048, 1024, 16, 64
NCORES = 8
HP = 2            # heads per core
SC = 512          # query chunk (columns of scoresT)
NSC = S // SC     # 4 chunks per batch
TT = 128          # key tile
NTT = S // TT     # 16 key tiles per batch
EO = E // 128     # 8 contraction chunks


class SafeTileContext(tile.TileContext):
    """TileContext with the tail drain's sem waits split across multiple
    Drain instructions - walrus here rejects >1 sync wait per instruction."""

    MAX_DRAIN_WAITS = 1

    def _drain_and_barrier(self, tick_clock, wait_clock):
        nc = self.nc
        drain_inst = nc.sync.drain()
        wait_clock.add_sem_waits(
            drain_inst.ins, ScopedClock({None: tick_clock.global_clock})
        )
        si = drain_inst.ins.sync_info
        if si is not None and si.on_wait and len(si.on_wait) > self.MAX_DRAIN_WAITS:
            waits = list(si.on_wait)
            si.on_wait = waits[: self.MAX_DRAIN_WAITS]
            drain_inst.ins.sync_info = si
            for i in range(self.MAX_DRAIN_WAITS, len(waits), self.MAX_DRAIN_WAITS):
                extra = nc.sync.drain()
                extra.ins.sync_info = mybir.SyncInfo(
                    on_wait=waits[i : i + self.MAX_DRAIN_WAITS], on_update=[]
                )
        nc.all_engine_barrier()
        assert self.sems is not None
        popped = nc._tile_sem_poison_stack.pop()
        assert popped is self._sem_poison
        nc.clear_and_free_semaphores(list(self.sems.allocated().values()))
        nc.all_engine_barrier()


def split_sync_waits(nc, maxw=1):
    """Hoist excess sync waits onto same-engine NoOps inserted just before
    the over-limit instruction (this container's walrus allows ~1)."""
    n_split = 0
    for f in nc.m.functions:
        for blk in f.blocks:
            out = []
            for ins in blk.instructions:
                si = ins.sync_info
                if si is not None and si.on_wait and len(si.on_wait) > maxw:
                    waits = list(si.on_wait)
                    extra, keep = waits[:-maxw], waits[-maxw:]
                    for j in range(0, len(extra), maxw):
                        nop = mybir.InstNoOp(
                            name=f"{ins.name}-wsplit{j}", ins=[], outs=[]
                        )
                        nop.engine = ins.engine
                        nop.sync_info = mybir.SyncInfo(
                            on_wait=extra[j : j + maxw], on_update=[]
                        )
                        out.append(nop)
                    si.on_wait = keep
                    ins.sync_info = si
                    n_split += 1
                out.append(ins)
            blk.instructions = out
    return n_split


def build_bass():
    nc = bass.Bass()
    x_d = nc.dram_tensor("x", [B, E, S], BF, kind="ExternalInput")  # pre-transposed on host
    wq_d = nc.dram_tensor("wq", [E, HP * DH], BF, kind="ExternalInput")
    wk_d = nc.dram_tensor("wk", [E, HP * DH], BF, kind="ExternalInput")
    wv_d = nc.dram_tensor("wv", [E, HP * DH], BF, kind="ExternalInput")
    bqkv_d = nc.dram_tensor("bqkv", [128, 3], F32, kind="ExternalInput")
    wo_d = nc.dram_tensor("wo", [HP * DH, E], BF, kind="ExternalInput")
    z_d = nc.dram_tensor("zpart", [B, S, E], BF, kind="ExternalOutput")

    with SafeTileContext(nc) as tc:
        with (
            tc.tile_pool(name="const", bufs=1) as constp,
            tc.tile_pool(name="xt", bufs=3) as xt_p,
            tc.tile_pool(name="qkv", bufs=2) as qkv_p,
            tc.tile_pool(name="vpool", bufs=2) as v_p,
            tc.tile_pool(name="vt", bufs=2) as vt_p,
            tc.tile_pool(name="expt", bufs=4) as exp_p,
            tc.tile_pool(name="oraw", bufs=4) as oraw_p,
            tc.tile_pool(name="onrm", bufs=2) as onrm_p,
            tc.tile_pool(name="osb", bufs=2) as osb_p,
            tc.tile_pool(name="den", bufs=2) as den_p,
            tc.tile_pool(name="zsb", bufs=4) as z_p,
            tc.tile_pool(name="ps_qk", bufs=1, space="PSUM") as ps_qk,
            tc.tile_pool(name="ps_po", bufs=2, space="PSUM") as ps_po,
            tc.tile_pool(name="ps_pw", bufs=2, space="PSUM") as ps_pw,
            tc.tile_pool(name="ps_tr", bufs=2, space="PSUM") as ps_tr,
        ):
            ident = constp.tile([128, 128], F32)
            make_identity(nc, ident)
            ident_bf = constp.tile([128, 128], BF)
            nc.vector.tensor_copy(out=ident_bf, in_=ident)

            # weights: [ei, eo, h*dh] stationary layout, bf16 straight from HBM
            w_sbs = []
            for nm, wd in (("wq", wq_d), ("wk", wk_d), ("wv", wv_d)):
                w_sb = constp.tile([128, EO, 128], BF, name=f"{nm}_sb")
                nc.sync.dma_start(
                    w_sb, wd.rearrange("(eo ei) d -> ei eo d", ei=128)
                )
                w_sbs.append(w_sb)
            wq_sb, wk_sb, wv_sb = w_sbs

            wo_sb = constp.tile([128, E], BF)
            nc.sync.dma_start(wo_sb, wo_d[:, :])

            bias_sb = constp.tile([128, 3], F32)
            nc.sync.dma_start(bias_sb, bqkv_d[:, :])

            ones_sb = constp.tile([128, 1], F32)
            nc.vector.memset(ones_sb, 1.0)

            # PE warmup: ~3us of matmuls so HAM reaches K=8/8 while the
            # first x/weight DMAs land.
            warm_sb = constp.tile([128, 512], BF)
            nc.vector.memset(warm_sb, 0.0)
            warm_ps = ps_pw.tile([128, 512], F32, tag="pw", name="warm_ps")
            for i in range(12):
                nc.tensor.matmul(
                    warm_ps, lhsT=warm_sb[:, 0:128], rhs=warm_sb,
                    start=(i == 0), stop=(i == 11),
                )

            batch_tiles = {}
            chunk_po = {}

            def head_slices(b, c):
                """Emission closures: x-transposes + q/k/v projections for
                chunk (b, c). 7 slices to interleave into attention."""
                s0 = c * SC

                def make_tiles():
                    qT = qkv_p.tile([128, S], BF, tag="qT", name=f"qT_{b}")
                    kT = qkv_p.tile([128, S], BF, tag="kT", name=f"kT_{b}")
                    v_sb = v_p.tile(
                        [128, HP, NTT, DH + 1], BF, tag="v", name=f"v_{b}"
                    )
                    nc.vector.tensor_copy(
                        out=v_sb[:, :, :, DH : DH + 1],
                        in_=ones_sb.to_broadcast([128, HP, NTT, 1]),
                    )
                    batch_tiles[b] = (qT, kT, v_sb)

                xt = xt_p.tile([128, EO, SC], BF, tag="xt", name=f"xt_{b}_{c}")

                def dma_slice():
                    if c == 0:
                        make_tiles()
                    nc.sync.dma_start(
                        xt,
                        x_d[b, :, s0 : s0 + SC].rearrange(
                            "(eo ei) s -> ei eo s", ei=128
                        ),
                    )

                def proj_slice(kind):
                    def run():
                        qT, kT, v_sb = batch_tiles[b]
                        w_sb = {"q": wq_sb, "k": wk_sb, "v": wv_sb}[kind]
                        psp = ps_pw.tile([128, 512], F32, tag="pw", name="psp")
                        for eo in range(EO):
                            nc.tensor.matmul(
                                psp,
                                lhsT=w_sb[:, eo, :],
                                rhs=xt[:, eo, :],
                                start=(eo == 0),
                                stop=(eo == EO - 1),
                            )
                        col = {"q": 0, "k": 1, "v": 2}[kind]
                        bias_ap = bias_sb[:, col : col + 1]
                        if kind == "q":
                            nc.scalar.activation(
                                qT[:, s0 : s0 + SC], psp, AF.Identity, bias=bias_ap
                            )
                        elif kind == "k":
                            nc.scalar.activation(
                                kT[:, s0 : s0 + SC], psp, AF.Identity, bias=bias_ap
                            )
                        else:
                            vt = vt_p.tile([128, SC], BF, tag="vt", name="vt")
                            nc.vector.tensor_scalar_add(vt, psp, bias_ap)
                            for vg in range(2):
                                pstv = ps_tr.tile(
                                    [128, 2, 128], BF, tag="tr", name="pstv"
                                )
                                for tl in range(2):
                                    nc.tensor.transpose(
                                        pstv[:, tl, :],
                                        vt[
                                            :,
                                            (vg * 2 + tl) * 128
                                            : (vg * 2 + tl + 1) * 128,
                                        ],
                                        ident_bf,
                                    )
                                pv4 = pstv.rearrange(
                                    "p a (q b) -> p a q b", q=HP
                                )
                                for h in range(HP):
                                    nc.vector.tensor_copy(
                                        out=v_sb[
                                            :, h,
                                            c * 4 + vg * 2 : c * 4 + vg * 2 + 2,
                                            0:DH,
                                        ],
                                        in_=pv4[:, :, h, :],
                                    )
                    return run

                return [dma_slice, proj_slice("q"), proj_slice("k"), proj_slice("v")]

            def z_slices(b, c):
                """Normalization + output projection for chunk (b, c),
                consuming the saved PV accumulators. 5 slices."""
                s0 = c * SC
                box = {}

                def norm_slice():
                    po = chunk_po.pop((b, c))
                    ot_raw = [
                        oraw_p.tile([128, SC], BF, tag="or", name=f"or{h}_{b}_{c}")
                        for h in range(HP)
                    ]
                    for h in range(HP):
                        nc.vector.tensor_copy(
                            out=ot_raw[h][0 : DH + 1, :], in_=po[h][:, :]
                        )
                    dnat = ps_tr.tile([128, 8, 128], BF, tag="tr", name="dnat")
                    for st in range(4):
                        for h in range(HP):
                            nc.tensor.transpose(
                                dnat[:, st * 2 + h, :],
                                ot_raw[h][:, st * 128 : (st + 1) * 128],
                                ident_bf,
                            )
                    den_sb = den_p.tile([128, 8, 1], F32, tag="den", name="den_sb")
                    nc.vector.reciprocal(den_sb, dnat[:, :, DH : DH + 1])
                    onorm = onrm_p.tile([128, 8, DH], BF, tag="on", name="onorm")
                    nc.vector.scalar_tensor_tensor(
                        out=onorm,
                        in0=dnat[:, :, 0:DH],
                        scalar=1.0,
                        in1=den_sb.to_broadcast([128, 8, DH]),
                        op0=ALU.mult,
                        op1=ALU.mult,
                    )
                    box["onorm"] = onorm

                def pack_slice():
                    onorm = box["onorm"]
                    otp = ps_tr.tile([128, 4, 128], BF, tag="tr", name="otp")
                    for st in range(4):
                        nc.tensor.transpose(
                            otp[:, st, :], onorm[:, st * 2 : (st + 1) * 2, :],
                            ident_bf,
                        )
                    ot_sb = osb_p.tile([128, 4, 128], BF, tag="os", name="ot_sb")
                    nc.vector.tensor_copy(out=ot_sb, in_=otp)
                    box["ot_sb"] = ot_sb

                def wo_slice(st):
                    def run():
                        ot_sb = box["ot_sb"]
                        for ec in range(E // 512):
                            pz = ps_pw.tile([128, 512], F32, tag="pw", name="pz")
                            nc.tensor.matmul(
                                pz,
                                lhsT=ot_sb[:, st, :],
                                rhs=wo_sb[:, ec * 512 : (ec + 1) * 512],
                                start=True,
                                stop=True,
                            )
                            zt = z_p.tile([128, 512], BF, tag="z", name="zt")
                            nc.vector.tensor_copy(out=zt, in_=pz)
                            nc.sync.dma_start(
                                z_d[
                                    b,
                                    s0 + st * 128 : s0 + (st + 1) * 128,
                                    ec * 512 : (ec + 1) * 512,
                                ],
                                zt,
                            )
                    return run

                def wo01():
                    pack_slice()
                    wo_slice(0)()

                return [norm_slice, wo01, wo_slice(1), wo_slice(2), wo_slice(3)]

            def att_emit(b, c, fillers):
                """QK -> exp -> mask -> PV key-tile stream for chunk (b, c),
                with filler slices interleaved to keep the PE dense."""
                s0 = c * SC
                nkt = 4 * (c + 1)
                qT, kT, v_sb = batch_tiles[b]
                po = [
                    ps_po.tile([DH + 1, 512], F32, tag="po", name=f"po{h}_{b}_{c}")
                    for h in range(HP)
                ]
                chunk_po[(b, c)] = po

                def emit_pv(tt, et):
                    j = tt - 4 * c
                    cs = 128 * j if j > 0 else 0
                    for h in range(HP):
                        nc.tensor.matmul(
                            po[h][:, cs:512],
                            lhsT=v_sb[:, h, tt, :],
                            rhs=et[:, h, cs:512],
                            start=(tt == 0),
                            stop=(tt == nkt - 1),
                            skip_group_check=True,
                        )

                prev = None
                emitted = 0
                for tt in range(nkt):
                    j = tt - 4 * c
                    cs = 128 * j if j > 0 else 0
                    pss = ps_qk.tile([128, HP, 512], F32, tag="qk", name="pss")
                    for h in range(HP):
                        nc.tensor.matmul(
                            pss[:, h, cs:512],
                            lhsT=kT[h * DH : (h + 1) * DH, tt * TT : (tt + 1) * TT],
                            rhs=qT[h * DH : (h + 1) * DH, s0 + cs : s0 + SC],
                            start=True,
                            stop=True,
                        )
                    et = exp_p.tile([128, HP, SC], BF, tag="et", name="et")
                    nc.scalar.activation(
                        et[:, :, cs:512], pss[:, :, cs:512], AF.Exp, scale=0.125
                    )
                    if j >= 0:
                        # diagonal tile: zero where t > s (also zero-fills
                        # the skipped columns below cs)
                        nc.gpsimd.affine_select(
                            out=et,
                            in_=et,
                            pattern=[[0, HP], [1, SC]],
                            compare_op=ALU.is_ge,
                            fill=0.0,
                            base=-128 * j,
                            channel_multiplier=-1,
                        )
                    if prev is not None:
                        emit_pv(*prev)
                    prev = (tt, et)
                    target = (len(fillers) * (tt + 1)) // nkt
                    while emitted < target:
                        fillers[emitted]()
                        emitted += 1
                emit_pv(*prev)
                while emitted < len(fillers):
                    fillers[emitted]()
                    emitted += 1

            chunks = [(b, c) for b in range(B) for c in range(NSC)]
            for sl in head_slices(*chunks[0]):
                sl()
            for i, (b, c) in enumerate(chunks):
                # hoisted ahead of the key-tile loop: the previous chunk's
                # normalization (frees its PV banks before this chunk's
                # first PV) and the next chunk's xt DMA (prefetch)
                pre = []
                fillers = []
                if i >= 1:
                    fillers += z_slices(*chunks[i - 1])
                if i + 1 < len(chunks):
                    hs = head_slices(*chunks[i + 1])
                    pre.append(hs[0])
                    fillers += hs[1:]
                for p in pre:
                    p()
                att_emit(b, c, fillers)
            for sl in z_slices(*chunks[-1]):
                sl()

    split_sync_waits(nc, maxw=1)
    return nc


_NC_CACHE = None
_LAST_RESULTS = None


def _get_nc():
    global _NC_CACHE
    if _NC_CACHE is None:
        _NC_CACHE = build_bass()
    return _NC_CACHE


def kernel(x, Wq, Wk, Wv, bq, bk, bv, Wo, bo):
    nc = _get_nc()
    bf16 = ml_dtypes.bfloat16
    x_bf = np.ascontiguousarray(
        np.asarray(x, dtype=np.float32).astype(bf16).transpose(0, 2, 1)
    )
    in_maps = []
    for cidx in range(NCORES):
        h0 = HP * cidx
        bias = np.stack(
            [
                np.asarray(bq[h0 : h0 + HP], np.float32).reshape(HP * DH),
                np.asarray(bk[h0 : h0 + HP], np.float32).reshape(HP * DH),
                np.asarray(bv[h0 : h0 + HP], np.float32).reshape(HP * DH),
            ],
            axis=1,
        )
        wq_p = np.concatenate(
            [np.asarray(Wq[h0 + h], np.float32) for h in range(HP)], axis=1
        ).astype(bf16)
        wk_p = np.concatenate(
            [np.asarray(Wk[h0 + h], np.float32) for h in range(HP)], axis=1
        ).astype(bf16)
        wv_p = np.concatenate(
            [np.asarray(Wv[h0 + h], np.float32) for h in range(HP)], axis=1
        ).astype(bf16)
        in_maps.append(
            {
                "x": x_bf,
                "wq": np.ascontiguousarray(wq_p),
                "wk": np.ascontiguousarray(wk_p),
                "wv": np.ascontiguousarray(wv_p),
                "bqkv": np.ascontiguousarray(bias, np.float32),
                "wo": np.ascontiguousarray(
                    np.asarray(Wo[h0 * DH : (h0 + HP) * DH, :], np.float32).astype(
                        bf16
                    )
                ),
            }
        )
    kw = {}
    if os.environ.get("KERNEL_TRACE"):
        kw = dict(trace=True, tmpdir=os.environ.get("KERNEL_TRACE_DIR") or None)
    res = run_bass_kernel_spmd(nc, in_maps, core_ids=list(range(NCORES)), **kw)
    global _LAST_RESULTS
    _LAST_RESULTS = res
    acc = np.zeros((B, S, E), np.float64)
    for r in res.results:
        acc += r["zpart"].astype(np.float64)
    acc += np.asarray(bo, np.float64)[None, None, :]
    return acc.astype(np.float32)


# revision 21
# speedup vs baseline: 1.2891x; 1.0619x over previous
"""Multi-head causal self-attention (B=4, S=2048, E=1024, H=16, Dh=64) on 8
Trainium2 NeuronCores.

Sharding: tensor-parallel over heads - 2 heads per core. Each core computes
q/k/v projections, causal attention and its slice of the output projection
(rows of Wo for its heads); the host sums the 8 partial outputs and adds bo.

v2 design (vs the fp32r baseline):
  - bf16 everywhere on the PE (inputs host-cast): transposes run at 1
    cycle/row (vs 2 for fp32), DMA traffic and DVE copies are halved, and
    PSUM transpose staging fits in half-banks. PSUM accumulation stays fp32.
  - QK pair per key tile issued back-to-back as row-tiled matmuls (head 0 in
    PE rows 0-63, head 1 in rows 64-127, via base_partition auto tiling) so
    the two heads' scores co-run on the array; their outputs land in one
    [128, 2, 512] PSUM pair-tile and are exp'd by a single ACT instruction.
  - causal: key tiles above the diagonal are skipped entirely; on diagonal
    tiles the QK/exp streams are restricted to the unmasked query columns
    and gpsimd affine_select zero-fills the masked region of the exp tile.
  - softmax denominators ride as a ones-column appended to v (PV output row
    64); attention outputs are transposed to natural [token, dh] layout,
    normalized there with per-partition reciprocals, and transposed back
    packed [2*64, token] so the output projection runs as single K=128
    matmuls (full PE density, no post-scaling of z needed).
  - emission interleaves the next chunk's x-transposes/projections and the
    previous chunk's normalization + output projection into the attention
    key-tile stream, so the PE never sees a long transpose-only or
    DMA-wait window (transposes don't count as PE-busy for the HAM clock
    gate; an idle window re-throttles the PE to 1.2 GHz).
  - a short warmup matmul burst at kernel start brings the PE out of the
    cold 1.2 GHz state while the first DMAs land.

All PE transposes use full 128-partition operands: partial-partition
transposes interleaved with other PE work crash the device
(NRT_EXEC_UNIT_UNRECOVERABLE).
"""

import os

import ml_dtypes
import numpy as np

import concourse.bass as bass
import concourse.mybir as mybir
import concourse.tile as tile
from concourse.vector_clock import ScopedClock
from concourse.masks import make_identity
from concourse.bass_utils import run_bass_kernel_spmd

F32 = mybir.dt.float32
BF = mybir.dt.bfloat16
AF = mybir.ActivationFunctionType
ALU = mybir.AluOpType

B, S, E, H, DH = 4, 2048, 1024, 16, 64
NCORES = 8
HP = 2            # heads per core
SC = 512          # query chunk (columns of scoresT)
NSC = S // SC     # 4 chunks per batch
TT = 128          # key tile
NTT = S // TT     # 16 key tiles per batch
EO = E // 128     # 8 contraction chunks


class SafeTileContext(tile.TileContext):
    """TileContext with the tail drain's sem waits split across multiple
    Drain instructions - walrus here rejects >1 sync wait per instruction."""

    MAX_DRAIN_WAITS = 1

    def _drain_and_barrier(self, tick_clock, wait_clock):
        nc = self.nc
        drain_inst = nc.sync.drain()
        wait_clock.add_sem_waits(
            drain_inst.ins, ScopedClock({None: tick_clock.global_clock})
        )
        si = drain_inst.ins.sync_info
        if si is not None and si.on_wait and len(si.on_wait) > self.MAX_DRAIN_WAITS:
            waits = list(si.on_wait)
            si.on_wait = waits[: self.MAX_DRAIN_WAITS]
            drain_inst.ins.sync_info = si
            for i in range(self.MAX_DRAIN_WAITS, len(waits), self.MAX_DRAIN_WAITS):
                extra = nc.sync.drain()
                extra.ins.sync_info = mybir.SyncInfo(
                    on_wait=waits[i : i + self.MAX_DRAIN_WAITS], on_update=[]
                )
        nc.all_engine_barrier()
        assert self.sems is not None
        popped = nc._tile_sem_poison_stack.pop()
        assert popped is self._sem_poison
        nc.clear_and_free_semaphores(list(self.sems.allocated().values()))
        nc.all_engine_barrier()


def split_sync_waits(nc, maxw=1):
    """Hoist excess sync waits onto same-engine NoOps inserted just before
    the over-limit instruction (this container's walrus allows ~1)."""
    n_split = 0
    for f in nc.m.functions:
        for blk in f.blocks:
            out = []
            for ins in blk.instructions:
                si = ins.sync_info
                if si is not None and si.on_wait and len(si.on_wait) > maxw:
                    waits = list(si.on_wait)
                    extra, keep = waits[:-maxw], waits[-maxw:]
                    for j in range(0, len(extra), maxw):
                        nop = mybir.InstNoOp(
                            name=f"{ins.name}-wsplit{j}", ins=[], outs=[]
                        )
                        nop.engine = ins.engine
                        nop.sync_info = mybir.SyncInfo(
                            on_wait=extra[j : j + maxw], on_update=[]
                        )
                        out.append(nop)
                    si.on_wait = keep
                    ins.sync_info = si
                    n_split += 1
                out.append(ins)
            blk.instructions = out
    return n_split


def build_bass():
    nc = bass.Bass()
    x_d = nc.dram_tensor("x", [B, E, S], BF, kind="ExternalInput")  # pre-transposed on host
    wq_d = nc.dram_tensor("wq", [E, HP * DH], BF, kind="ExternalInput")
    wk_d = nc.dram_tensor("wk", [E, HP * DH], BF, kind="ExternalInput")
    wv_d = nc.dram_tensor("wv", [E, HP * DH], BF, kind="ExternalInput")
    bqkv_d = nc.dram_tensor("bqkv", [128, 3], F32, kind="ExternalInput")
    wo_d = nc.dram_tensor("wo", [HP * DH, E], BF, kind="ExternalInput")
    z_d = nc.dram_tensor("zpart", [B, S, E], BF, kind="ExternalOutput")

    with SafeTileContext(nc) as tc:
        with (
            tc.tile_pool(name="const", bufs=1) as constp,
            tc.tile_pool(name="xt", bufs=3) as xt_p,
            tc.tile_pool(name="qkv", bufs=2) as qkv_p,
            tc.tile_pool(name="vpool", bufs=2) as v_p,
            tc.tile_pool(name="vt", bufs=2) as vt_p,
            tc.tile_pool(name="expt", bufs=4) as exp_p,
            tc.tile_pool(name="oraw", bufs=4) as oraw_p,
            tc.tile_pool(name="onrm", bufs=2) as onrm_p,
            tc.tile_pool(name="osb", bufs=2) as osb_p,
            tc.tile_pool(name="den", bufs=2) as den_p,
            tc.tile_pool(name="zsb", bufs=4) as z_p,
            tc.tile_pool(name="ps_qk", bufs=2, space="PSUM") as ps_qk,
            tc.tile_pool(name="ps_po", bufs=2, space="PSUM") as ps_po,
            tc.tile_pool(name="ps_tr", bufs=2, space="PSUM") as ps_tr,
        ):
            ident = constp.tile([128, 128], F32)
            make_identity(nc, ident)
            ident_bf = constp.tile([128, 128], BF)
            nc.vector.tensor_copy(out=ident_bf, in_=ident)

            # weights: [ei, eo, h*dh] stationary layout, bf16 straight from HBM
            w_sbs = []
            for nm, wd in (("wq", wq_d), ("wk", wk_d), ("wv", wv_d)):
                w_sb = constp.tile([128, EO, 128], BF, name=f"{nm}_sb")
                nc.sync.dma_start(
                    w_sb, wd.rearrange("(eo ei) d -> ei eo d", ei=128)
                )
                w_sbs.append(w_sb)
            wq_sb, wk_sb, wv_sb = w_sbs

            wo_sb = constp.tile([128, E], BF)
            nc.sync.dma_start(wo_sb, wo_d[:, :])

            bias_sb = constp.tile([128, 3], F32)
            nc.sync.dma_start(bias_sb, bqkv_d[:, :])

            ones_sb = constp.tile([128, 1], F32)
            nc.vector.memset(ones_sb, 1.0)

            # PE warmup: ~3us of matmuls so HAM reaches K=8/8 while the
            # first x/weight DMAs land.
            warm_sb = constp.tile([128, 512], BF)
            nc.vector.memset(warm_sb, 0.0)
            warm_ps = ps_tr.tile([128, 512], F32, tag="tr", name="warm_ps")
            for i in range(12):
                nc.tensor.matmul(
                    warm_ps, lhsT=warm_sb[:, 0:128], rhs=warm_sb,
                    start=(i == 0), stop=(i == 11),
                )

            batch_tiles = {}
            chunk_po = {}

            def head_slices(b, c):
                """Emission closures: x-transposes + q/k/v projections for
                chunk (b, c). 7 slices to interleave into attention."""
                s0 = c * SC

                def make_tiles():
                    qT = qkv_p.tile([128, S], BF, tag="qT", name=f"qT_{b}")
                    kT = qkv_p.tile([128, S], BF, tag="kT", name=f"kT_{b}")
                    v_sb = v_p.tile(
                        [128, HP, NTT, DH + 1], BF, tag="v", name=f"v_{b}"
                    )
                    nc.vector.tensor_copy(
                        out=v_sb[:, :, :, DH : DH + 1],
                        in_=ones_sb.to_broadcast([128, HP, NTT, 1]),
                    )
                    batch_tiles[b] = (qT, kT, v_sb)

                xt = xt_p.tile([128, EO, SC], BF, tag="xt", name=f"xt_{b}_{c}")

                def dma_slice():
                    if c == 0:
                        make_tiles()
                    nc.sync.dma_start(
                        xt,
                        x_d[b, :, s0 : s0 + SC].rearrange(
                            "(eo ei) s -> ei eo s", ei=128
                        ),
                    )

                def proj_slice(kind):
                    def run():
                        qT, kT, v_sb = batch_tiles[b]
                        w_sb = {"q": wq_sb, "k": wk_sb, "v": wv_sb}[kind]
                        psp = ps_tr.tile([128, 512], F32, tag="tr", name="psp")
                        for eo in range(EO):
                            nc.tensor.matmul(
                                psp,
                                lhsT=w_sb[:, eo, :],
                                rhs=xt[:, eo, :],
                                start=(eo == 0),
                                stop=(eo == EO - 1),
                            )
                        col = {"q": 0, "k": 1, "v": 2}[kind]
                        bias_ap = bias_sb[:, col : col + 1]
                        if kind == "q":
                            nc.scalar.activation(
                                qT[:, s0 : s0 + SC], psp, AF.Identity, bias=bias_ap
                            )
                        elif kind == "k":
                            nc.scalar.activation(
                                kT[:, s0 : s0 + SC], psp, AF.Identity, bias=bias_ap
                            )
                        else:
                            vt = vt_p.tile([128, SC], BF, tag="vt", name="vt")
                            nc.vector.tensor_scalar_add(vt, psp, bias_ap)
                            for vg in range(2):
                                pstv = ps_tr.tile(
                                    [128, 2, 128], BF, tag="tr", name="pstv"
                                )
                                for tl in range(2):
                                    nc.tensor.transpose(
                                        pstv[:, tl, :],
                                        vt[
                                            :,
                                            (vg * 2 + tl) * 128
                                            : (vg * 2 + tl + 1) * 128,
                                        ],
                                        ident_bf,
                                    )
                                pv4 = pstv.rearrange(
                                    "p a (q b) -> p a q b", q=HP
                                )
                                for h in range(HP):
                                    nc.vector.tensor_copy(
                                        out=v_sb[
                                            :, h,
                                            c * 4 + vg * 2 : c * 4 + vg * 2 + 2,
                                            0:DH,
                                        ],
                                        in_=pv4[:, :, h, :],
                                    )
                    return run

                return [dma_slice, proj_slice("q"), proj_slice("k"), proj_slice("v")]

            def z_slices(b, c):
                """Normalization + output projection for chunk (b, c),
                consuming the saved PV accumulators. 5 slices."""
                s0 = c * SC
                box = {}

                def norm_slice():
                    po = chunk_po.pop((b, c))
                    ot_raw = [
                        oraw_p.tile([128, SC], BF, tag="or", name=f"or{h}_{b}_{c}")
                        for h in range(HP)
                    ]
                    for h in range(HP):
                        nc.vector.tensor_copy(
                            out=ot_raw[h][0 : DH + 1, :], in_=po[h][:, :]
                        )
                    dnat = ps_tr.tile([128, 8, 128], BF, tag="tr", name="dnat")
                    for st in range(4):
                        for h in range(HP):
                            nc.tensor.transpose(
                                dnat[:, st * 2 + h, :],
                                ot_raw[h][:, st * 128 : (st + 1) * 128],
                                ident_bf,
                            )
                    den_sb = den_p.tile([128, 8, 1], F32, tag="den", name="den_sb")
                    nc.vector.reciprocal(den_sb, dnat[:, :, DH : DH + 1])
                    onorm = onrm_p.tile([128, 8, DH], BF, tag="on", name="onorm")
                    nc.vector.scalar_tensor_tensor(
                        out=onorm,
                        in0=dnat[:, :, 0:DH],
                        scalar=1.0,
                        in1=den_sb.to_broadcast([128, 8, DH]),
                        op0=ALU.mult,
                        op1=ALU.mult,
                    )
                    box["onorm"] = onorm

                def pack_slice():
                    onorm = box["onorm"]
                    otp = ps_tr.tile([128, 4, 128], BF, tag="tr", name="otp")
                    for st in range(4):
                        nc.tensor.transpose(
                            otp[:, st, :], onorm[:, st * 2 : (st + 1) * 2, :],
                            ident_bf,
                        )
                    ot_sb = osb_p.tile([128, 4, 128], BF, tag="os", name="ot_sb")
                    nc.vector.tensor_copy(out=ot_sb, in_=otp)
                    box["ot_sb"] = ot_sb

                def wo_slice(st):
                    def run():
                        ot_sb = box["ot_sb"]
                        for ec in range(E // 512):
                            pz = ps_tr.tile([128, 512], F32, tag="tr", name="pz")
                            nc.tensor.matmul(
                                pz,
                                lhsT=ot_sb[:, st, :],
                                rhs=wo_sb[:, ec * 512 : (ec + 1) * 512],
                                start=True,
                                stop=True,
                            )
                            zt = z_p.tile([128, 512], BF, tag="z", name="zt")
                            nc.vector.tensor_copy(out=zt, in_=pz)
                            nc.sync.dma_start(
                                z_d[
                                    b,
                                    s0 + st * 128 : s0 + (st + 1) * 128,
                                    ec * 512 : (ec + 1) * 512,
                                ],
                                zt,
                            )
                    return run

                def wo01():
                    pack_slice()
                    wo_slice(0)()

                return [norm_slice, wo01, wo_slice(1), wo_slice(2), wo_slice(3)]

            def att_emit(b, c, fillers):
                """QK -> exp -> mask -> PV key-tile stream for chunk (b, c),
                with filler slices interleaved to keep the PE dense."""
                s0 = c * SC
                nkt = 4 * (c + 1)
                qT, kT, v_sb = batch_tiles[b]
                po = [
                    ps_po.tile([DH + 1, 512], F32, tag="po", name=f"po{h}_{b}_{c}")
                    for h in range(HP)
                ]
                chunk_po[(b, c)] = po

                def emit_pv(tt, et):
                    j = tt - 4 * c
                    cs = 128 * j if j > 0 else 0
                    for h in range(HP):
                        nc.tensor.matmul(
                            po[h][:, cs:512],
                            lhsT=v_sb[:, h, tt, :],
                            rhs=et[:, h, cs:512],
                            start=(tt == 0),
                            stop=(tt == nkt - 1),
                            skip_group_check=True,
                        )

                prev = None
                emitted = 0
                for tt in range(nkt):
                    j = tt - 4 * c
                    cs = 128 * j if j > 0 else 0
                    pss = ps_qk.tile([128, HP, 512], F32, tag="qk", name="pss")
                    for h in range(HP):
                        nc.tensor.matmul(
                            pss[:, h, cs:512],
                            lhsT=kT[h * DH : (h + 1) * DH, tt * TT : (tt + 1) * TT],
                            rhs=qT[h * DH : (h + 1) * DH, s0 + cs : s0 + SC],
                            start=True,
                            stop=True,
                        )
                    et = exp_p.tile([128, HP, SC], BF, tag="et", name="et")
                    nc.scalar.activation(
                        et[:, :, cs:512], pss[:, :, cs:512], AF.Exp, scale=0.125
                    )
                    if j >= 0:
                        # diagonal tile: zero where t > s (also zero-fills
                        # the skipped columns below cs)
                        nc.gpsimd.affine_select(
                            out=et,
                            in_=et,
                            pattern=[[0, HP], [1, SC]],
                            compare_op=ALU.is_ge,
                            fill=0.0,
                            base=-128 * j,
                            channel_multiplier=-1,
                        )
                    if prev is not None:
                        emit_pv(*prev)
                    prev = (tt, et)
                    target = (len(fillers) * (tt + 1)) // nkt
                    while emitted < target:
                        fillers[emitted]()
                        emitted += 1
                emit_pv(*prev)
                while emitted < len(fillers):
                    fillers[emitted]()
                    emitted += 1

            chunks = [(b, c) for b in range(B) for c in range(NSC)]
            for sl in head_slices(*chunks[0]):
                sl()
            for i, (b, c) in enumerate(chunks):
                # hoisted ahead of the key-tile loop: the previous chunk's
                # normalization (frees its PV banks before this chunk's
                # first PV) and the next chunk's xt DMA (prefetch)
                pre = []
                fillers = []
                if i >= 1:
                    fillers += z_slices(*chunks[i - 1])
                if i + 1 < len(chunks):
                    hs = head_slices(*chunks[i + 1])
                    pre.append(hs[0])
                    fillers += hs[1:]
                for p in pre:
                    p()
                att_emit(b, c, fillers)
            for sl in z_slices(*chunks[-1]):
                sl()

    split_sync_waits(nc, maxw=1)
    return nc


_NC_CACHE = None
_LAST_RESULTS = None


def _get_nc():
    global _NC_CACHE
    if _NC_CACHE is None:
        _NC_CACHE = build_bass()
    return _NC_CACHE


def kernel(x, Wq, Wk, Wv, bq, bk, bv, Wo, bo):
    nc = _get_nc()
    bf16 = ml_dtypes.bfloat16
    x_bf = np.ascontiguousarray(
        np.asarray(x, dtype=np.float32).astype(bf16).transpose(0, 2, 1)
    )
    in_maps = []
    for cidx in range(NCORES):
        h0 = HP * cidx
        bias = np.stack(
            [
                np.asarray(bq[h0 : h0 + HP], np.float32).reshape(HP * DH),
                np.asarray(bk[h0 : h0 + HP], np.float32).reshape(HP * DH),
                np.asarray(bv[h0 : h0 + HP], np.float32).reshape(HP * DH),
            ],
            axis=1,
        )
        wq_p = np.concatenate(
            [np.asarray(Wq[h0 + h], np.float32) for h in range(HP)], axis=1
        ).astype(bf16)
        wk_p = np.concatenate(
            [np.asarray(Wk[h0 + h], np.float32) for h in range(HP)], axis=1
        ).astype(bf16)
        wv_p = np.concatenate(
            [np.asarray(Wv[h0 + h], np.float32) for h in range(HP)], axis=1
        ).astype(bf16)
        in_maps.append(
            {
                "x": x_bf,
                "wq": np.ascontiguousarray(wq_p),
                "wk": np.ascontiguousarray(wk_p),
                "wv": np.ascontiguousarray(wv_p),
                "bqkv": np.ascontiguousarray(bias, np.float32),
                "wo": np.ascontiguousarray(
                    np.asarray(Wo[h0 * DH : (h0 + HP) * DH, :], np.float32).astype(
                        bf16
                    )
                ),
            }
        )
    kw = {}
    if os.environ.get("KERNEL_TRACE"):
        kw = dict(trace=True, tmpdir=os.environ.get("KERNEL_TRACE_DIR") or None)
    res = run_bass_kernel_spmd(nc, in_maps, core_ids=list(range(NCORES)), **kw)
    global _LAST_RESULTS
    _LAST_RESULTS = res
    acc = np.zeros((B, S, E), np.float64)
    for r in res.results:
        acc += r["zpart"].astype(np.float64)
    acc += np.asarray(bo, np.float64)[None, None, :]
    return acc.astype(np.float32)
